# revision 10
# baseline (speedup 1.0000x reference)
"""Trainium2 Bass kernel for nn_AdaptiveNoiseModule0 (8-core data parallel).

Math (per sample b):
  feat   = mean_L(bottleneck_feat[b])                  # [512]
  h      = relu(LN(feat @ W1 + b1) * ln_g + ln_b)
  a, be  = sigmoid(h @ W2 + b2) * 1.5e-4               # per-sample scalars
  sigma  = sqrt(a * x_bg + be)       (variance >= be ~ 7e-5 >> 1e-7 floor)
  n      = gaussian_blur(noise, 7, 0.05) * 1.5 == noise * 1.5
           (the 7-tap sigma=0.05 kernel underflows to the delta in f32)
  msoft  = gaussian_blur(mask, 21, 5.0)                # separable 21-tap
  out    = x_bg + n * sigma * msoft

Device mapping: one sample per NeuronCore. The 21-tap separable blur runs on
the TensorEngine as banded matmuls (transpose -> W-blur -> transpose -> H-blur)
in bf16. The streaming phase computes sigma' = sqrt(2.25*a*x + 2.25*be) on
ScalarE (the 1.5 noise gain folded into the radicand) and the elementwise
chain on VectorE, with the noise term in bf16 (it contributes ~1% of the
output) and the dominant x_bg term kept in f32.
"""

import numpy as np
import ml_dtypes

B, C, H, W = 8, 3, 1024, 1024
L, D = 1024, 512
DH = 128
MAX_ALPHA = 0.00015
MAX_BETA = 0.00015
LN_EPS = 1e-5
KSIZE, SIGMA = 21, 5.0
R = KSIZE // 2
N_CORES = 8

F32 = None  # filled at build time (mybir handles)
BF16 = None

_CACHE = {}


def _blur_weights():
    """Banded blur matrices for the 128-row-tile matmul formulation.

    out_tile[m] = Wprev.T @ prev_chunk[118:128] + Wmid.T @ mid_chunk
                  + Wnext.T @ next_chunk[0:10]
    """
    g = np.exp(-(np.linspace(-R, R, KSIZE, dtype=np.float32) ** 2)
               / (2.0 * np.float32(SIGMA) ** 2))
    g = (g / g.sum()).astype(np.float32)

    wmid = np.zeros((128, 128), np.float32)
    for kk in range(128):
        for m in range(max(0, kk - R), min(128, kk + R + 1)):
            wmid[kk, m] = g[kk - m + R]
    # full 128-row matrices (PE requires operand base partition 0/32/64)
    wprev = np.zeros((128, 128), np.float32)
    for kk in range(118, 128):
        for m in range(0, kk - 118 + 1):
            wprev[kk, m] = g[kk - 118 - m]
    wnext = np.zeros((128, 128), np.float32)
    for kk in range(10):
        for m in range(118 + kk, 128):
            wnext[kk, m] = g[138 + kk - m]
    bf = ml_dtypes.bfloat16
    return wmid.astype(bf), wprev.astype(bf), wnext.astype(bf)


def _build():
    import concourse.bass as bass
    import concourse.tile as tile
    from concourse import bacc, mybir

    f32 = mybir.dt.float32
    bf16 = mybir.dt.bfloat16
    AF = mybir.ActivationFunctionType
    AX = mybir.AxisListType

    nc = bacc.Bacc("TRN2", target_bir_lowering=False, debug=False)

    # ---- per-core parameters (shard = one sample) ----
    x_d = nc.declare_dram_parameter("x", [C * H, W], f32, isOutput=False)
    nz_d = nc.declare_dram_parameter("nz", [C * H, W], bf16, isOutput=False)
    mk_d = nc.declare_dram_parameter("mk", [H, W], bf16, isOutput=False)
    bft_d = nc.declare_dram_parameter("bft", [D, L], f32, isOutput=False)  # pre-transposed
    w1_d = nc.declare_dram_parameter("w1s", [D, DH], f32, isOutput=False)  # W1 / L
    b1_d = nc.declare_dram_parameter("b1c", [DH, 1], f32, isOutput=False)
    lng_d = nc.declare_dram_parameter("lngr", [1, DH], f32, isOutput=False)
    lnb_d = nc.declare_dram_parameter("lnbr", [1, DH], f32, isOutput=False)
    w2_d = nc.declare_dram_parameter("w2", [DH, 2], f32, isOutput=False)
    b2_d = nc.declare_dram_parameter("b2r", [1, 2], f32, isOutput=False)
    wmid_d = nc.declare_dram_parameter("wmid", [128, 128], bf16, isOutput=False)
    wprev_d = nc.declare_dram_parameter("wprev", [128, 128], bf16, isOutput=False)
    wnext_d = nc.declare_dram_parameter("wnext", [128, 128], bf16, isOutput=False)
    idf_d = nc.declare_dram_parameter("idf", [128, 128], f32, isOutput=False)
    ones_d = nc.declare_dram_parameter("onesr", [1, 128], f32, isOutput=False)

    out_d = nc.declare_dram_parameter("out", [C * H, W], f32, isOutput=True)
    ab_d = nc.declare_dram_parameter("out_ab", [1, 2], f32, isOutput=True)

    from contextlib import ExitStack

    with tile.TileContext(nc) as tc, ExitStack() as es:
        def pool(name, bufs, space=None):
            kw = {"space": space} if space else {}
            return es.enter_context(tc.tile_pool(name=name, bufs=bufs, **kw))

        PSUM = bass.MemorySpace.PSUM
        consts = pool("consts", 1)
        headp = pool("headp", 2)
        featp = pool("featp", 4)
        w1p = pool("w1p", 4)
        bftp = pool("bftp", 2)
        xtp = pool("xtp", 8)
        vtp = pool("vtp", 8)
        vp = pool("vp", 8)
        msp = pool("msp", 8)
        xp = pool("xp", 8)
        sgp = pool("sgp", 4)
        nbp = pool("nbp", 8)
        t1p = pool("t1p", 3)
        t2p = pool("t2p", 3)
        op = pool("op", 8)
        ps_mm = pool("ps_mm", 3, PSUM)
        ps_tr = pool("ps_tr", 3, PSUM)
        ps_hd = pool("ps_hd", 2, PSUM)
        if True:
            # ---------- constant loads ----------
            idf = consts.tile([128, 128], f32)
            nc.gpsimd.dma_start(idf[:], idf_d[:])
            wmid = consts.tile([128, 128], bf16)
            nc.gpsimd.dma_start(wmid[:], wmid_d[:])
            wprev = consts.tile([128, 128], bf16)
            nc.gpsimd.dma_start(wprev[:], wprev_d[:])
            wnext = consts.tile([128, 128], bf16)
            nc.gpsimd.dma_start(wnext[:], wnext_d[:])
            ones_r = consts.tile([1, 128], f32)
            nc.gpsimd.dma_start(ones_r[:], ones_d[:])

            # ---------- mask DMA, transposed on the fly (xbar) ----------
            xt_t = []
            for j in range(8):
                xt = xtp.tile([128, W], bf16, tag="xt")
                nc.scalar.dma_start(xt[:], mk_d[:, 128 * j:128 * (j + 1)],
                                    transpose=True)
                xt_t.append(xt)

            # ---------- param head ----------
            feat_c = []
            for dc in range(4):
                bt = bftp.tile([128, L], f32, tag="bft")
                nc.gpsimd.dma_start(bt[:], bft_d[128 * dc:128 * (dc + 1), :])
                fc = featp.tile([128, 1], f32, tag="feat")
                nc.vector.reduce_sum(fc[:], bt[:], axis=AX.X)
                feat_c.append(fc)
            w1_t = []
            for dc in range(4):
                wt = w1p.tile([128, DH], f32, tag="w1")
                nc.gpsimd.dma_start(wt[:], w1_d[128 * dc:128 * (dc + 1), :])
                w1_t.append(wt)
            b1_t = headp.tile([DH, 1], f32)
            nc.gpsimd.dma_start(b1_t[:], b1_d[:])
            lng_t = headp.tile([1, DH], f32)
            nc.gpsimd.dma_start(lng_t[:], lng_d[:])
            lnb_t = headp.tile([1, DH], f32)
            nc.gpsimd.dma_start(lnb_t[:], lnb_d[:])
            w2_t = headp.tile([DH, 2], f32)
            nc.gpsimd.dma_start(w2_t[:], w2_d[:])
            b2_t = headp.tile([1, 2], f32)
            nc.gpsimd.dma_start(b2_t[:], b2_d[:])

            h_ps = ps_hd.tile([DH, 1], f32, tag="head")
            for dc in range(4):
                nc.tensor.matmul(h_ps[:], w1_t[dc][:], feat_c[dc][:],
                                 start=(dc == 0), stop=(dc == 3))
            h_sb = headp.tile([DH, 1], f32)
            nc.vector.tensor_add(h_sb[:], h_ps[:], b1_t[:])

            ht_ps = ps_hd.tile([1, DH], f32, tag="head")
            nc.tensor.transpose(ht_ps[:], h_sb[:], idf[:])
            ht = headp.tile([1, DH], f32)
            nc.scalar.copy(ht[:], ht_ps[:])

            musum = headp.tile([1, 1], f32)
            nc.vector.reduce_sum(musum[:], ht[:], axis=AX.X)
            mu = headp.tile([1, 1], f32)
            nc.scalar.mul(mu[:], musum[:], 1.0 / DH)
            ctr = headp.tile([1, DH], f32)
            nc.vector.tensor_scalar_sub(ctr[:], ht[:], mu[:])
            sq = headp.tile([1, DH], f32)
            nc.vector.tensor_mul(sq[:], ctr[:], ctr[:])
            vsum = headp.tile([1, 1], f32)
            nc.vector.reduce_sum(vsum[:], sq[:], axis=AX.X)
            eps_t = headp.tile([1, 1], f32)
            nc.vector.memset(eps_t[:], LN_EPS)
            sstd = headp.tile([1, 1], f32)
            nc.scalar.activation(sstd[:], vsum[:], AF.Sqrt,
                                 bias=eps_t[:], scale=1.0 / DH)
            rstd = headp.tile([1, 1], f32)
            nc.vector.reciprocal(rstd[:], sstd[:])
            hn = headp.tile([1, DH], f32)
            nc.vector.tensor_scalar_mul(hn[:], ctr[:], rstd[:])
            hn2 = headp.tile([1, DH], f32)
            nc.vector.tensor_mul(hn2[:], hn[:], lng_t[:])
            hn3 = headp.tile([1, DH], f32)
            nc.vector.tensor_add(hn3[:], hn2[:], lnb_t[:])
            hr = headp.tile([1, DH], f32)
            nc.vector.tensor_scalar_max(hr[:], hn3[:], 0.0)

            hcol_ps = ps_hd.tile([DH, 1], f32, tag="head")
            nc.tensor.transpose(hcol_ps[:], hr[:], idf[0:1, 0:1])
            hcol = headp.tile([DH, 1], f32)
            nc.scalar.copy(hcol[:], hcol_ps[:])

            p_ps = ps_hd.tile([1, 2], f32, tag="head")
            nc.tensor.matmul(p_ps[:], hcol[:], w2_t[:], start=True, stop=True)
            pb = headp.tile([1, 2], f32)
            nc.vector.tensor_add(pb[:], p_ps[:], b2_t[:])
            sg = headp.tile([1, 2], f32)
            nc.scalar.activation(sg[:], pb[:], AF.Sigmoid)
            ab_out = headp.tile([1, 2], f32)
            nc.scalar.mul(ab_out[:], sg[:], MAX_ALPHA)
            nc.scalar.dma_start(ab_d[:], ab_out[:])
            # 2.25 folds the n = 1.5*noise gain into the radicand
            sg2 = headp.tile([1, 2], f32)
            nc.scalar.mul(sg2[:], sg[:], 2.25 * MAX_ALPHA)
            ab_ps = ps_hd.tile([128, 2], f32, tag="head")
            nc.tensor.matmul(ab_ps[:], ones_r[:], sg2[:], start=True, stop=True)
            alpha_b = headp.tile([128, 1], f32)
            nc.scalar.copy(alpha_b[:], ab_ps[:, 0:1])
            beta_b = headp.tile([128, 1], f32)
            nc.scalar.copy(beta_b[:], ab_ps[:, 1:2])

            # ---------- mask blur: W-blur -> T2 -> H-blur ----------
            def banded(dst_tiles, src_tiles, dst_pool, tag):
                for j in range(8):
                    dst = dst_pool.tile([128, W], bf16, tag=tag)
                    for half in range(2):
                        bp = ps_mm.tile([128, 512], f32, tag="mm")
                        sl = slice(512 * half, 512 * (half + 1))
                        mms = []
                        if j > 0:
                            mms.append((wprev, src_tiles[j - 1][:, sl]))
                        mms.append((wmid, src_tiles[j][:, sl]))
                        if j < 7:
                            mms.append((wnext, src_tiles[j + 1][:, sl]))
                        for i, (wt, rhs) in enumerate(mms):
                            nc.tensor.matmul(bp[:], wt[:], rhs,
                                             start=(i == 0), stop=(i == len(mms) - 1))
                        nc.scalar.copy(dst[:, sl], bp[:])
                    dst_tiles.append(dst)

            # W-blur on XT: VT = A @ XT  (= (X A)^T)
            vt_t = []
            banded(vt_t, xt_t, vtp, "vt")

            # T2: V = VT^T via xbar DMA transposes (bf16)
            v_t = []
            for r in range(8):
                v = vp.tile([128, W], bf16, tag="v")
                for m in range(8):
                    nc.scalar.dma_start(
                        v[:, 128 * m:128 * (m + 1)],
                        vt_t[m][:, 128 * r:128 * (r + 1)], transpose=True)
                v_t.append(v)

            # H-blur: msoft = A @ V
            ms_t = []
            banded(ms_t, v_t, msp, "ms")

            # ---------- streaming phase ----------
            for r in range(8):
                for c in range(C):
                    off = c * H + 128 * r
                    xt_ = xp.tile([128, W], f32, tag="x")
                    nc.sync.dma_start(xt_[:], x_d[off:off + 128, :])
                    nb_ = nbp.tile([128, W], bf16, tag="nb")
                    nc.sync.dma_start(nb_[:], nz_d[off:off + 128, :])
                    sg_ = sgp.tile([128, W], bf16, tag="sg")
                    nc.scalar.activation(sg_[:], xt_[:], AF.Sqrt,
                                         bias=beta_b[:], scale=alpha_b[:])
                    t1_ = t1p.tile([128, W], bf16, tag="t1")
                    nc.gpsimd.tensor_mul(t1_[:], nb_[:], sg_[:])
                    t2_ = t2p.tile([128, W], bf16, tag="t2")
                    nc.vector.tensor_mul(t2_[:], t1_[:], ms_t[r][:])
                    o_ = op.tile([128, W], f32, tag="o")
                    nc.vector.tensor_add(o_[:], xt_[:], t2_[:])
                    nc.scalar.dma_start(out_d[off:off + 128, :], o_[:])

    nc.compile()
    return nc


def _get_nc():
    if "nc" not in _CACHE:
        _CACHE["nc"] = _build()
    return _CACHE["nc"]


def kernel(x_bg, bottleneck_feat, mask, noise, W1, b1, ln_g, ln_b, W2, b2):
    from concourse.bass_utils import run_bass_kernel_spmd

    nc = _get_nc()
    bf = ml_dtypes.bfloat16

    wmid, wprev, wnext = _blur_weights()
    idf = np.eye(128, dtype=np.float32)
    ones_r = np.ones((1, 128), np.float32)
    shared = {
        "w1s": np.ascontiguousarray(W1.astype(np.float32) / np.float32(L)),
        "b1c": np.ascontiguousarray(b1.astype(np.float32).reshape(DH, 1)),
        "lngr": np.ascontiguousarray(ln_g.astype(np.float32).reshape(1, DH)),
        "lnbr": np.ascontiguousarray(ln_b.astype(np.float32).reshape(1, DH)),
        "w2": np.ascontiguousarray(W2.astype(np.float32)),
        "b2r": np.ascontiguousarray(b2.astype(np.float32).reshape(1, 2)),
        "wmid": wmid, "wprev": wprev, "wnext": wnext,
        "idf": idf, "onesr": ones_r,
    }
    in_maps = []
    for i in range(N_CORES):
        in_maps.append({
            "x": np.ascontiguousarray(x_bg[i].reshape(C * H, W), dtype=np.float32),
            "nz": np.asarray(noise[i].reshape(C * H, W), dtype=bf),
            "mk": np.asarray(mask[i, 0], dtype=bf),
            "bft": np.ascontiguousarray(bottleneck_feat[i].T, dtype=np.float32),
            **shared,
        })

    res = run_bass_kernel_spmd(nc, in_maps, core_ids=list(range(N_CORES)))
    noisy = np.stack([res.results[i]["out"].reshape(C, H, W)
                      for i in range(N_CORES)])
    ab = np.stack([res.results[i]["out_ab"].reshape(2)
                   for i in range(N_CORES)])
    alpha = np.ascontiguousarray(ab[:, 0]).reshape(B, 1, 1, 1)
    beta = np.ascontiguousarray(ab[:, 1]).reshape(B, 1, 1, 1)
    return noisy, alpha, beta


# revision 11
# speedup vs baseline: 1.1542x; 1.1542x over previous
"""Trainium2 Bass kernel for nn_AdaptiveNoiseModule0 (8-core data parallel).

Math (per sample b):
  feat   = mean_L(bottleneck_feat[b])                  # [512]
  h      = relu(LN(feat @ W1 + b1) * ln_g + ln_b)
  a, be  = sigmoid(h @ W2 + b2) * 1.5e-4               # per-sample scalars
  sigma  = sqrt(a * x_bg + be)       (variance >= be ~ 7e-5 >> 1e-7 floor)
  n      = gaussian_blur(noise, 7, 0.05) * 1.5 == noise * 1.5
           (the 7-tap sigma=0.05 kernel underflows to the delta in f32)
  msoft  = gaussian_blur(mask, 21, 5.0)                # separable 21-tap
  out    = x_bg + n * sigma * msoft

Device mapping: one sample per NeuronCore. The 21-tap separable blur runs on
the TensorEngine as banded matmuls (transpose -> W-blur -> transpose -> H-blur)
in bf16. The streaming phase computes sigma' = sqrt(2.25*a*x + 2.25*be) on
ScalarE (the 1.5 noise gain folded into the radicand) and the elementwise
chain on VectorE, with the noise term in bf16 (it contributes ~1% of the
output) and the dominant x_bg term kept in f32.
"""

import numpy as np
import ml_dtypes

B, C, H, W = 8, 3, 1024, 1024
L, D = 1024, 512
DH = 128
MAX_ALPHA = 0.00015
MAX_BETA = 0.00015
LN_EPS = 1e-5
KSIZE, SIGMA = 21, 5.0
R = KSIZE // 2
N_CORES = 8

F32 = None  # filled at build time (mybir handles)
BF16 = None

_CACHE = {}


def _blur_weights():
    """Banded blur matrices for the 128-row-tile matmul formulation.

    out_tile[m] = Wprev.T @ prev_chunk[118:128] + Wmid.T @ mid_chunk
                  + Wnext.T @ next_chunk[0:10]
    """
    g = np.exp(-(np.linspace(-R, R, KSIZE, dtype=np.float32) ** 2)
               / (2.0 * np.float32(SIGMA) ** 2))
    g = (g / g.sum()).astype(np.float32)

    wmid = np.zeros((128, 128), np.float32)
    for kk in range(128):
        for m in range(max(0, kk - R), min(128, kk + R + 1)):
            wmid[kk, m] = g[kk - m + R]
    # full 128-row matrices (PE requires operand base partition 0/32/64)
    wprev = np.zeros((128, 128), np.float32)
    for kk in range(118, 128):
        for m in range(0, kk - 118 + 1):
            wprev[kk, m] = g[kk - 118 - m]
    wnext = np.zeros((128, 128), np.float32)
    for kk in range(10):
        for m in range(118 + kk, 128):
            wnext[kk, m] = g[138 + kk - m]
    bf = ml_dtypes.bfloat16
    return wmid.astype(bf), wprev.astype(bf), wnext.astype(bf)


def _build():
    import concourse.bass as bass
    import concourse.tile as tile
    from concourse import bacc, mybir

    f32 = mybir.dt.float32
    bf16 = mybir.dt.bfloat16
    AF = mybir.ActivationFunctionType
    AX = mybir.AxisListType

    nc = bacc.Bacc("TRN2", target_bir_lowering=False, debug=False)

    # ---- per-core parameters (shard = one sample) ----
    x_d = nc.declare_dram_parameter("x", [C * H, W], f32, isOutput=False)
    nz_d = nc.declare_dram_parameter("nz", [C * H, W], bf16, isOutput=False)
    mk_d = nc.declare_dram_parameter("mk", [H, W], bf16, isOutput=False)
    bft_d = nc.declare_dram_parameter("bft", [D, L], f32, isOutput=False)  # pre-transposed
    w1_d = nc.declare_dram_parameter("w1s", [D, DH], f32, isOutput=False)  # W1 / L
    b1_d = nc.declare_dram_parameter("b1c", [DH, 1], f32, isOutput=False)
    lng_d = nc.declare_dram_parameter("lngr", [1, DH], f32, isOutput=False)
    lnb_d = nc.declare_dram_parameter("lnbr", [1, DH], f32, isOutput=False)
    w2_d = nc.declare_dram_parameter("w2", [DH, 2], f32, isOutput=False)
    b2_d = nc.declare_dram_parameter("b2r", [1, 2], f32, isOutput=False)
    wmid_d = nc.declare_dram_parameter("wmid", [128, 128], bf16, isOutput=False)
    wprev_d = nc.declare_dram_parameter("wprev", [128, 128], bf16, isOutput=False)
    wnext_d = nc.declare_dram_parameter("wnext", [128, 128], bf16, isOutput=False)
    idf_d = nc.declare_dram_parameter("idf", [128, 128], f32, isOutput=False)
    idb_d = nc.declare_dram_parameter("idb", [128, 128], bf16, isOutput=False)
    ones_d = nc.declare_dram_parameter("onesr", [1, 128], f32, isOutput=False)

    out_d = nc.declare_dram_parameter("out", [C * H, W], f32, isOutput=True)
    ab_d = nc.declare_dram_parameter("out_ab", [1, 2], f32, isOutput=True)

    from contextlib import ExitStack

    with tile.TileContext(nc) as tc, ExitStack() as es:
        def pool(name, bufs, space=None):
            kw = {"space": space} if space else {}
            return es.enter_context(tc.tile_pool(name=name, bufs=bufs, **kw))

        PSUM = bass.MemorySpace.PSUM
        consts = pool("consts", 1)
        headp = pool("headp", 2)
        featp = pool("featp", 4)
        w1p = pool("w1p", 4)
        bftp = pool("bftp", 2)
        xtp = pool("xtp", 8)
        vtp = pool("vtp", 8)
        vp = pool("vp", 8)
        msp = pool("msp", 8)
        xp = pool("xp", 8)
        sgp = pool("sgp", 4)
        nbp = pool("nbp", 8)
        t1p = pool("t1p", 3)
        t2p = pool("t2p", 3)
        op = pool("op", 8)
        ps_mm = pool("ps_mm", 3, PSUM)
        ps_tr = pool("ps_tr", 3, PSUM)
        ps_hd = pool("ps_hd", 2, PSUM)
        if True:
            # ---------- constant loads ----------
            idf = consts.tile([128, 128], f32)
            nc.gpsimd.dma_start(idf[:], idf_d[:])
            idb = consts.tile([128, 128], bf16)
            nc.gpsimd.dma_start(idb[:], idb_d[:])
            wmid = consts.tile([128, 128], bf16)
            nc.gpsimd.dma_start(wmid[:], wmid_d[:])
            wprev = consts.tile([128, 128], bf16)
            nc.gpsimd.dma_start(wprev[:], wprev_d[:])
            wnext = consts.tile([128, 128], bf16)
            nc.gpsimd.dma_start(wnext[:], wnext_d[:])
            ones_r = consts.tile([1, 128], f32)
            nc.gpsimd.dma_start(ones_r[:], ones_d[:])

            # ---------- mask DMA, transposed on the fly (xbar) ----------
            xt_t = []
            for j in range(8):
                xt = xtp.tile([128, W], bf16, tag="xt")
                nc.scalar.dma_start(xt[:], mk_d[:, 128 * j:128 * (j + 1)],
                                    transpose=True)
                xt_t.append(xt)

            # ---------- param head ----------
            feat_c = []
            for dc in range(4):
                bt = bftp.tile([128, L], f32, tag="bft")
                nc.gpsimd.dma_start(bt[:], bft_d[128 * dc:128 * (dc + 1), :])
                fc = featp.tile([128, 1], f32, tag="feat")
                nc.vector.reduce_sum(fc[:], bt[:], axis=AX.X)
                feat_c.append(fc)
            w1_t = []
            for dc in range(4):
                wt = w1p.tile([128, DH], f32, tag="w1")
                nc.gpsimd.dma_start(wt[:], w1_d[128 * dc:128 * (dc + 1), :])
                w1_t.append(wt)
            b1_t = headp.tile([DH, 1], f32)
            nc.gpsimd.dma_start(b1_t[:], b1_d[:])
            lng_t = headp.tile([1, DH], f32)
            nc.gpsimd.dma_start(lng_t[:], lng_d[:])
            lnb_t = headp.tile([1, DH], f32)
            nc.gpsimd.dma_start(lnb_t[:], lnb_d[:])
            w2_t = headp.tile([DH, 2], f32)
            nc.gpsimd.dma_start(w2_t[:], w2_d[:])
            b2_t = headp.tile([1, 2], f32)
            nc.gpsimd.dma_start(b2_t[:], b2_d[:])

            h_ps = ps_hd.tile([DH, 1], f32, tag="head")
            for dc in range(4):
                nc.tensor.matmul(h_ps[:], w1_t[dc][:], feat_c[dc][:],
                                 start=(dc == 0), stop=(dc == 3))
            h_sb = headp.tile([DH, 1], f32)
            nc.vector.tensor_add(h_sb[:], h_ps[:], b1_t[:])

            ht_ps = ps_hd.tile([1, DH], f32, tag="head")
            nc.tensor.transpose(ht_ps[:], h_sb[:], idf[:])
            ht = headp.tile([1, DH], f32)
            nc.scalar.copy(ht[:], ht_ps[:])

            musum = headp.tile([1, 1], f32)
            nc.vector.reduce_sum(musum[:], ht[:], axis=AX.X)
            mu = headp.tile([1, 1], f32)
            nc.scalar.mul(mu[:], musum[:], 1.0 / DH)
            ctr = headp.tile([1, DH], f32)
            nc.vector.tensor_scalar_sub(ctr[:], ht[:], mu[:])
            sq = headp.tile([1, DH], f32)
            nc.vector.tensor_mul(sq[:], ctr[:], ctr[:])
            vsum = headp.tile([1, 1], f32)
            nc.vector.reduce_sum(vsum[:], sq[:], axis=AX.X)
            eps_t = headp.tile([1, 1], f32)
            nc.vector.memset(eps_t[:], LN_EPS)
            sstd = headp.tile([1, 1], f32)
            nc.scalar.activation(sstd[:], vsum[:], AF.Sqrt,
                                 bias=eps_t[:], scale=1.0 / DH)
            rstd = headp.tile([1, 1], f32)
            nc.vector.reciprocal(rstd[:], sstd[:])
            hn = headp.tile([1, DH], f32)
            nc.vector.tensor_scalar_mul(hn[:], ctr[:], rstd[:])
            hn2 = headp.tile([1, DH], f32)
            nc.vector.tensor_mul(hn2[:], hn[:], lng_t[:])
            hn3 = headp.tile([1, DH], f32)
            nc.vector.tensor_add(hn3[:], hn2[:], lnb_t[:])
            hr = headp.tile([1, DH], f32)
            nc.vector.tensor_scalar_max(hr[:], hn3[:], 0.0)

            hcol_ps = ps_hd.tile([DH, 1], f32, tag="head")
            nc.tensor.transpose(hcol_ps[:], hr[:], idf[0:1, 0:1])
            hcol = headp.tile([DH, 1], f32)
            nc.scalar.copy(hcol[:], hcol_ps[:])

            p_ps = ps_hd.tile([1, 2], f32, tag="head")
            nc.tensor.matmul(p_ps[:], hcol[:], w2_t[:], start=True, stop=True)
            pb = headp.tile([1, 2], f32)
            nc.vector.tensor_add(pb[:], p_ps[:], b2_t[:])
            sg = headp.tile([1, 2], f32)
            nc.scalar.activation(sg[:], pb[:], AF.Sigmoid)
            ab_out = headp.tile([1, 2], f32)
            nc.scalar.mul(ab_out[:], sg[:], MAX_ALPHA)
            nc.scalar.dma_start(ab_d[:], ab_out[:])
            # 2.25 folds the n = 1.5*noise gain into the radicand
            sg2 = headp.tile([1, 2], f32)
            nc.scalar.mul(sg2[:], sg[:], 2.25 * MAX_ALPHA)
            ab_ps = ps_hd.tile([128, 2], f32, tag="head")
            nc.tensor.matmul(ab_ps[:], ones_r[:], sg2[:], start=True, stop=True)
            alpha_b = headp.tile([128, 1], f32)
            nc.scalar.copy(alpha_b[:], ab_ps[:, 0:1])
            beta_b = headp.tile([128, 1], f32)
            nc.scalar.copy(beta_b[:], ab_ps[:, 1:2])

            # ---------- mask blur: W-blur -> T2 -> H-blur ----------
            def banded(dst_tiles, src_tiles, dst_pool, tag):
                for j in range(8):
                    dst = dst_pool.tile([128, W], bf16, tag=tag)
                    for half in range(2):
                        bp = ps_mm.tile([128, 512], f32, tag="mm")
                        sl = slice(512 * half, 512 * (half + 1))
                        mms = []
                        if j > 0:
                            mms.append((wprev, src_tiles[j - 1][:, sl]))
                        mms.append((wmid, src_tiles[j][:, sl]))
                        if j < 7:
                            mms.append((wnext, src_tiles[j + 1][:, sl]))
                        for i, (wt, rhs) in enumerate(mms):
                            nc.tensor.matmul(bp[:], wt[:], rhs,
                                             start=(i == 0), stop=(i == len(mms) - 1))
                        nc.scalar.copy(dst[:, sl], bp[:])
                    dst_tiles.append(dst)

            # W-blur on XT: VT = A @ XT  (= (X A)^T)
            vt_t = []
            banded(vt_t, xt_t, vtp, "vt")

            # T2: V = VT^T via PE transposes (bf16)
            v_t = []
            for r in range(8):
                v = vp.tile([128, W], bf16, tag="v")
                for half in range(2):
                    tp = ps_tr.tile([128, 512], bf16, tag="trb")
                    for q in range(4):
                        m = half * 4 + q
                        nc.tensor.transpose(
                            tp[:, 128 * q:128 * (q + 1)],
                            vt_t[m][:, 128 * r:128 * (r + 1)], idb[:])
                    nc.scalar.copy(v[:, 512 * half:512 * (half + 1)], tp[:])
                v_t.append(v)

            # H-blur: msoft = A @ V
            ms_t = []
            banded(ms_t, v_t, msp, "ms")

            # ---------- streaming phase ----------
            for r in range(8):
                for c in range(C):
                    off = c * H + 128 * r
                    xt_ = xp.tile([128, W], f32, tag="x")
                    nc.sync.dma_start(xt_[:], x_d[off:off + 128, :])
                    nb_ = nbp.tile([128, W], bf16, tag="nb")
                    nc.sync.dma_start(nb_[:], nz_d[off:off + 128, :])
                    sg_ = sgp.tile([128, W], bf16, tag="sg")
                    nc.scalar.activation(sg_[:], xt_[:], AF.Sqrt,
                                         bias=beta_b[:], scale=alpha_b[:])
                    t1_ = t1p.tile([128, W], bf16, tag="t1")
                    nc.gpsimd.tensor_mul(t1_[:], nb_[:], sg_[:])
                    t2_ = t2p.tile([128, W], bf16, tag="t2")
                    nc.vector.tensor_mul(t2_[:], t1_[:], ms_t[r][:])
                    o_ = op.tile([128, W], f32, tag="o")
                    nc.vector.tensor_add(o_[:], xt_[:], t2_[:])
                    nc.scalar.dma_start(out_d[off:off + 128, :], o_[:])

    nc.compile()
    return nc


def _get_nc():
    if "nc" not in _CACHE:
        _CACHE["nc"] = _build()
    return _CACHE["nc"]


def kernel(x_bg, bottleneck_feat, mask, noise, W1, b1, ln_g, ln_b, W2, b2):
    from concourse.bass_utils import run_bass_kernel_spmd

    nc = _get_nc()
    bf = ml_dtypes.bfloat16

    wmid, wprev, wnext = _blur_weights()
    idf = np.eye(128, dtype=np.float32)
    ones_r = np.ones((1, 128), np.float32)
    shared = {
        "w1s": np.ascontiguousarray(W1.astype(np.float32) / np.float32(L)),
        "b1c": np.ascontiguousarray(b1.astype(np.float32).reshape(DH, 1)),
        "lngr": np.ascontiguousarray(ln_g.astype(np.float32).reshape(1, DH)),
        "lnbr": np.ascontiguousarray(ln_b.astype(np.float32).reshape(1, DH)),
        "w2": np.ascontiguousarray(W2.astype(np.float32)),
        "b2r": np.ascontiguousarray(b2.astype(np.float32).reshape(1, 2)),
        "wmid": wmid, "wprev": wprev, "wnext": wnext,
        "idf": idf, "idb": idf.astype(bf), "onesr": ones_r,
    }
    in_maps = []
    for i in range(N_CORES):
        in_maps.append({
            "x": np.ascontiguousarray(x_bg[i].reshape(C * H, W), dtype=np.float32),
            "nz": np.asarray(noise[i].reshape(C * H, W), dtype=bf),
            "mk": np.asarray(mask[i, 0], dtype=bf),
            "bft": np.ascontiguousarray(bottleneck_feat[i].T, dtype=np.float32),
            **shared,
        })

    res = run_bass_kernel_spmd(nc, in_maps, core_ids=list(range(N_CORES)))
    noisy = np.stack([res.results[i]["out"].reshape(C, H, W)
                      for i in range(N_CORES)])
    ab = np.stack([res.results[i]["out_ab"].reshape(2)
                   for i in range(N_CORES)])
    alpha = np.ascontiguousarray(ab[:, 0]).reshape(B, 1, 1, 1)
    beta = np.ascontiguousarray(ab[:, 1]).reshape(B, 1, 1, 1)
    return noisy, alpha, beta


# revision 13
# speedup vs baseline: 1.4072x; 1.2192x over previous
"""Trainium2 Bass kernel for nn_AdaptiveNoiseModule0 (8-core data parallel).

Math (per sample b):
  feat   = mean_L(bottleneck_feat[b])                  # [512]
  h      = relu(LN(feat @ W1 + b1) * ln_g + ln_b)
  a, be  = sigmoid(h @ W2 + b2) * 1.5e-4               # per-sample scalars
  sigma  = sqrt(a * x_bg + be)       (variance >= be ~ 7e-5 >> 1e-7 floor)
  n      = gaussian_blur(noise, 7, 0.05) * 1.5 == noise * 1.5
           (the 7-tap sigma=0.05 kernel underflows to the delta in f32)
  msoft  = gaussian_blur(mask, 21, 5.0)                # separable 21-tap
  out    = x_bg + n * sigma * msoft

Device mapping: one sample per NeuronCore. The 21-tap separable blur runs on
the TensorEngine as banded matmuls plus PE transposes
(T1 -> W-blur -> T2 -> H-blur) in bf16. The streaming phase computes
sigma' = sqrt(2.25*a*x + 2.25*be) on ScalarE (the 1.5 noise gain folded into
the radicand), noise*sigma' on GpSimd, and the rest on VectorE, with the
noise term in bf16 (it contributes ~1% of the output) and the dominant x_bg
term kept in f32. noise and mask are pre-cast to bf16 on the host (identical
rounding to the on-device cast they'd otherwise get) to cut HBM traffic.
SBUF slots of early-phase tiles are tag-shared with late-phase tiles so the
x/noise prefetch can run deep while the blur chain completes.
"""

import numpy as np
import ml_dtypes

B, C, H, W = 8, 3, 1024, 1024
L, D = 1024, 512
DH = 128
MAX_ALPHA = 0.00015
MAX_BETA = 0.00015
LN_EPS = 1e-5
KSIZE, SIGMA = 21, 5.0
R = KSIZE // 2
N_CORES = 8

_CACHE = {}


def _blur_weights():
    """Banded blur matrices for the 128-row-tile matmul formulation.

    out_tile[t] = Wprev.T @ tile[t-1] + Wmid.T @ tile[t] + Wnext.T @ tile[t+1]
    (edge tiles skip the missing neighbor == zero padding).
    """
    g = np.exp(-(np.linspace(-R, R, KSIZE, dtype=np.float32) ** 2)
               / (2.0 * np.float32(SIGMA) ** 2))
    g = (g / g.sum()).astype(np.float32)

    wmid = np.zeros((128, 128), np.float32)
    for kk in range(128):
        for m in range(max(0, kk - R), min(128, kk + R + 1)):
            wmid[kk, m] = g[kk - m + R]
    wprev = np.zeros((128, 128), np.float32)
    for kk in range(118, 128):
        for m in range(0, kk - 118 + 1):
            wprev[kk, m] = g[kk - 118 - m]
    wnext = np.zeros((128, 128), np.float32)
    for kk in range(10):
        for m in range(118 + kk, 128):
            wnext[kk, m] = g[138 + kk - m]
    bf = ml_dtypes.bfloat16
    return wmid.astype(bf), wprev.astype(bf), wnext.astype(bf)


def _build():
    import concourse.bass as bass
    import concourse.tile as tile
    from concourse import bacc, mybir
    from contextlib import ExitStack

    f32 = mybir.dt.float32
    bf16 = mybir.dt.bfloat16
    AF = mybir.ActivationFunctionType
    AX = mybir.AxisListType

    nc = bacc.Bacc("TRN2", target_bir_lowering=False, debug=False)

    # ---- per-core parameters (shard = one sample) ----
    x_d = nc.declare_dram_parameter("x", [C * H, W], f32, isOutput=False)
    nz_d = nc.declare_dram_parameter("nz", [C * H, W], bf16, isOutput=False)
    mk_d = nc.declare_dram_parameter("mk", [H, W], bf16, isOutput=False)
    bft_d = nc.declare_dram_parameter("bft", [D, L], f32, isOutput=False)
    w1_d = nc.declare_dram_parameter("w1s", [D, DH], f32, isOutput=False)
    b1_d = nc.declare_dram_parameter("b1c", [DH, 1], f32, isOutput=False)
    lng_d = nc.declare_dram_parameter("lngr", [1, DH], f32, isOutput=False)
    lnb_d = nc.declare_dram_parameter("lnbr", [1, DH], f32, isOutput=False)
    w2_d = nc.declare_dram_parameter("w2", [DH, 2], f32, isOutput=False)
    b2_d = nc.declare_dram_parameter("b2r", [1, 2], f32, isOutput=False)
    wmid_d = nc.declare_dram_parameter("wmid", [128, 128], bf16, isOutput=False)
    wprev_d = nc.declare_dram_parameter("wprev", [128, 128], bf16, isOutput=False)
    wnext_d = nc.declare_dram_parameter("wnext", [128, 128], bf16, isOutput=False)
    idf_d = nc.declare_dram_parameter("idf", [128, 128], f32, isOutput=False)
    idb_d = nc.declare_dram_parameter("idb", [128, 128], bf16, isOutput=False)
    ones_d = nc.declare_dram_parameter("onesr", [1, 128], f32, isOutput=False)

    out_d = nc.declare_dram_parameter("out", [C * H, W], f32, isOutput=True)
    ab_d = nc.declare_dram_parameter("out_ab", [1, 2], f32, isOutput=True)

    with tile.TileContext(nc) as tc, ExitStack() as es:
        def pool(name, bufs, space=None):
            kw = {"space": space} if space else {}
            return es.enter_context(tc.tile_pool(name=name, bufs=bufs, **kw))

        PSUM = bass.MemorySpace.PSUM
        consts = pool("consts", 1)
        headp = pool("headp", 2)
        featp = pool("featp", 4)
        w1p = pool("w1p", 4)
        # bft tiles (early) share slots with phase-B out tiles (late):
        # both [128,1024] f32.
        bfo = pool("bfo", 8)
        # mask tiles (die after T1) share slots with sigma tiles (late)
        mks = pool("mks", 8)
        # xt tiles (die ~mid-blur) share slots with t2 tiles (born late)
        xt2 = pool("xt2", 8)
        vtp = pool("vtp", 8)
        vp = pool("vp", 8)
        msp = pool("msp", 8)
        xp = pool("xp", 9)
        nbp = pool("nbp", 10)
        t1p = pool("t1p", 6)
        ps_mm = pool("ps_mm", 3, PSUM)
        ps_tr = pool("ps_tr", 3, PSUM)
        ps_hd = pool("ps_hd", 2, PSUM)
        if True:
            # ---------- constant loads (gpsimd SWDGE ring) ----------
            idf = consts.tile([128, 128], f32)
            nc.gpsimd.dma_start(idf[:], idf_d[:])
            idb = consts.tile([128, 128], bf16)
            nc.gpsimd.dma_start(idb[:], idb_d[:])
            wmid = consts.tile([128, 128], bf16)
            nc.gpsimd.dma_start(wmid[:], wmid_d[:])
            wprev = consts.tile([128, 128], bf16)
            nc.gpsimd.dma_start(wprev[:], wprev_d[:])
            wnext = consts.tile([128, 128], bf16)
            nc.gpsimd.dma_start(wnext[:], wnext_d[:])
            ones_r = consts.tile([1, 128], f32)
            nc.gpsimd.dma_start(ones_r[:], ones_d[:])

            # ---------- mask DMA (blur critical path, sync ring) ----------
            mk_t = []
            for m in range(8):
                t = mks.tile([128, W], bf16, tag="mks")
                nc.sync.dma_start(t[:], mk_d[128 * m:128 * (m + 1), :])
                mk_t.append(t)

            # ---------- param head ----------
            feat_c = []
            for dc in range(4):
                bt = bfo.tile([128, L], f32, tag="bfo")
                nc.gpsimd.dma_start(bt[:], bft_d[128 * dc:128 * (dc + 1), :])
                fc = featp.tile([128, 1], f32, tag="feat")
                nc.vector.reduce_sum(fc[:], bt[:], axis=AX.X)
                feat_c.append(fc)
            w1_t = []
            for dc in range(4):
                wt = w1p.tile([128, DH], f32, tag="w1")
                nc.gpsimd.dma_start(wt[:], w1_d[128 * dc:128 * (dc + 1), :])
                w1_t.append(wt)
            b1_t = headp.tile([DH, 1], f32)
            nc.gpsimd.dma_start(b1_t[:], b1_d[:])
            lng_t = headp.tile([1, DH], f32)
            nc.gpsimd.dma_start(lng_t[:], lng_d[:])
            lnb_t = headp.tile([1, DH], f32)
            nc.gpsimd.dma_start(lnb_t[:], lnb_d[:])
            w2_t = headp.tile([DH, 2], f32)
            nc.gpsimd.dma_start(w2_t[:], w2_d[:])
            b2_t = headp.tile([1, 2], f32)
            nc.gpsimd.dma_start(b2_t[:], b2_d[:])

            h_ps = ps_hd.tile([DH, 1], f32, tag="head")
            for dc in range(4):
                nc.tensor.matmul(h_ps[:], w1_t[dc][:], feat_c[dc][:],
                                 start=(dc == 0), stop=(dc == 3))
            h_sb = headp.tile([DH, 1], f32)
            nc.vector.tensor_add(h_sb[:], h_ps[:], b1_t[:])

            ht_ps = ps_hd.tile([1, DH], f32, tag="head")
            nc.tensor.transpose(ht_ps[:], h_sb[:], idf[:])
            ht = headp.tile([1, DH], f32)
            nc.scalar.copy(ht[:], ht_ps[:])

            musum = headp.tile([1, 1], f32)
            nc.vector.reduce_sum(musum[:], ht[:], axis=AX.X)
            mu = headp.tile([1, 1], f32)
            nc.scalar.mul(mu[:], musum[:], 1.0 / DH)
            ctr = headp.tile([1, DH], f32)
            nc.vector.tensor_scalar_sub(ctr[:], ht[:], mu[:])
            sq = headp.tile([1, DH], f32)
            nc.vector.tensor_mul(sq[:], ctr[:], ctr[:])
            vsum = headp.tile([1, 1], f32)
            nc.vector.reduce_sum(vsum[:], sq[:], axis=AX.X)
            eps_t = headp.tile([1, 1], f32)
            nc.vector.memset(eps_t[:], LN_EPS)
            sstd = headp.tile([1, 1], f32)
            nc.scalar.activation(sstd[:], vsum[:], AF.Sqrt,
                                 bias=eps_t[:], scale=1.0 / DH)
            rstd = headp.tile([1, 1], f32)
            nc.vector.reciprocal(rstd[:], sstd[:])
            hn = headp.tile([1, DH], f32)
            nc.vector.tensor_scalar_mul(hn[:], ctr[:], rstd[:])
            hn2 = headp.tile([1, DH], f32)
            nc.vector.tensor_mul(hn2[:], hn[:], lng_t[:])
            hn3 = headp.tile([1, DH], f32)
            nc.vector.tensor_add(hn3[:], hn2[:], lnb_t[:])
            hr = headp.tile([1, DH], f32)
            nc.vector.tensor_scalar_max(hr[:], hn3[:], 0.0)

            hcol_ps = ps_hd.tile([DH, 1], f32, tag="head")
            nc.tensor.transpose(hcol_ps[:], hr[:], idf[0:1, 0:1])
            hcol = headp.tile([DH, 1], f32)
            nc.scalar.copy(hcol[:], hcol_ps[:])

            p_ps = ps_hd.tile([1, 2], f32, tag="head")
            nc.tensor.matmul(p_ps[:], hcol[:], w2_t[:], start=True, stop=True)
            pb = headp.tile([1, 2], f32)
            nc.vector.tensor_add(pb[:], p_ps[:], b2_t[:])
            sg = headp.tile([1, 2], f32)
            nc.scalar.activation(sg[:], pb[:], AF.Sigmoid)
            ab_out = headp.tile([1, 2], f32)
            nc.scalar.mul(ab_out[:], sg[:], MAX_ALPHA)
            nc.scalar.dma_start(ab_d[:], ab_out[:])
            # 2.25 folds the n = 1.5*noise gain into the radicand
            sg2 = headp.tile([1, 2], f32)
            nc.scalar.mul(sg2[:], sg[:], 2.25 * MAX_ALPHA)
            ab_ps = ps_hd.tile([128, 2], f32, tag="head")
            nc.tensor.matmul(ab_ps[:], ones_r[:], sg2[:], start=True, stop=True)
            alpha_b = headp.tile([128, 1], f32)
            nc.scalar.copy(alpha_b[:], ab_ps[:, 0:1])
            beta_b = headp.tile([128, 1], f32)
            nc.scalar.copy(beta_b[:], ab_ps[:, 1:2])

            # ---------- mask blur: T1 -> W-blur -> T2 -> H-blur ----------
            # T1: XT = mask^T (PE transposes, bf16)
            xt_t = []
            for j in range(8):
                xt = xt2.tile([128, W], bf16, tag="xt2")
                for half in range(2):
                    tp = ps_tr.tile([128, 512], bf16, tag="trb")
                    for q in range(4):
                        m = half * 4 + q
                        nc.tensor.transpose(
                            tp[:, 128 * q:128 * (q + 1)],
                            mk_t[m][:, 128 * j:128 * (j + 1)], idb[:])
                    nc.scalar.copy(xt[:, 512 * half:512 * (half + 1)], tp[:])
                xt_t.append(xt)

            def banded(dst_tiles, src_tiles, dst_pool, tag):
                for j in range(8):
                    dst = dst_pool.tile([128, W], bf16, tag=tag)
                    for half in range(2):
                        bp = ps_mm.tile([128, 512], f32, tag="mm")
                        sl = slice(512 * half, 512 * (half + 1))
                        mms = []
                        if j > 0:
                            mms.append((wprev, src_tiles[j - 1][:, sl]))
                        mms.append((wmid, src_tiles[j][:, sl]))
                        if j < 7:
                            mms.append((wnext, src_tiles[j + 1][:, sl]))
                        for i, (wt, rhs) in enumerate(mms):
                            nc.tensor.matmul(bp[:], wt[:], rhs,
                                             start=(i == 0), stop=(i == len(mms) - 1))
                        nc.vector.tensor_copy(dst[:, sl], bp[:])
                    dst_tiles.append(dst)

            # W-blur on XT: VT = A @ XT  (= (X A)^T)
            vt_t = []
            banded(vt_t, xt_t, vtp, "vt")

            # T2: V = VT^T (PE transposes, bf16)
            v_t = []
            for r in range(8):
                v = vp.tile([128, W], bf16, tag="v")
                for half in range(2):
                    tp = ps_tr.tile([128, 512], bf16, tag="trb")
                    for q in range(4):
                        m = half * 4 + q
                        nc.tensor.transpose(
                            tp[:, 128 * q:128 * (q + 1)],
                            vt_t[m][:, 128 * r:128 * (r + 1)], idb[:])
                    nc.scalar.copy(v[:, 512 * half:512 * (half + 1)], tp[:])
                v_t.append(v)

            # H-blur: msoft = A @ V
            ms_t = []
            banded(ms_t, v_t, msp, "ms")

            # ---------- streaming phase ----------
            for r in range(8):
                for c in range(C):
                    off = c * H + 128 * r
                    xt_ = xp.tile([128, W], f32, tag="x")
                    nc.sync.dma_start(xt_[:], x_d[off:off + 128, :])
                    nb_ = nbp.tile([128, W], bf16, tag="nb")
                    nc.sync.dma_start(nb_[:], nz_d[off:off + 128, :])
                    sg_ = mks.tile([128, W], bf16, tag="mks")
                    nc.scalar.activation(sg_[:], xt_[:], AF.Sqrt,
                                         bias=beta_b[:], scale=alpha_b[:])
                    t1_ = t1p.tile([128, W], bf16, tag="t1")
                    nc.gpsimd.tensor_mul(t1_[:], nb_[:], sg_[:])
                    t2_ = xt2.tile([128, W], bf16, tag="xt2")
                    nc.vector.tensor_mul(t2_[:], t1_[:], ms_t[r][:])
                    o_ = bfo.tile([128, W], f32, tag="bfo")
                    nc.vector.tensor_add(o_[:], xt_[:], t2_[:])
                    nc.scalar.dma_start(out_d[off:off + 128, :], o_[:])

    nc.compile()
    return nc


def _get_nc():
    if "nc" not in _CACHE:
        _CACHE["nc"] = _build()
    return _CACHE["nc"]


def kernel(x_bg, bottleneck_feat, mask, noise, W1, b1, ln_g, ln_b, W2, b2):
    from concourse.bass_utils import run_bass_kernel_spmd

    nc = _get_nc()
    bf = ml_dtypes.bfloat16

    wmid, wprev, wnext = _blur_weights()
    idf = np.eye(128, dtype=np.float32)
    shared = {
        "w1s": np.ascontiguousarray(W1.astype(np.float32) / np.float32(L)),
        "b1c": np.ascontiguousarray(b1.astype(np.float32).reshape(DH, 1)),
        "lngr": np.ascontiguousarray(ln_g.astype(np.float32).reshape(1, DH)),
        "lnbr": np.ascontiguousarray(ln_b.astype(np.float32).reshape(1, DH)),
        "w2": np.ascontiguousarray(W2.astype(np.float32)),
        "b2r": np.ascontiguousarray(b2.astype(np.float32).reshape(1, 2)),
        "wmid": wmid, "wprev": wprev, "wnext": wnext,
        "idf": idf, "idb": idf.astype(bf), "onesr": np.ones((1, 128), np.float32),
    }
    in_maps = []
    for i in range(N_CORES):
        in_maps.append({
            "x": np.ascontiguousarray(x_bg[i].reshape(C * H, W), dtype=np.float32),
            "nz": np.asarray(noise[i].reshape(C * H, W), dtype=bf),
            "mk": np.asarray(mask[i, 0], dtype=bf),
            "bft": np.ascontiguousarray(bottleneck_feat[i].T, dtype=np.float32),
            **shared,
        })

    res = run_bass_kernel_spmd(nc, in_maps, core_ids=list(range(N_CORES)))
    noisy = np.stack([res.results[i]["out"].reshape(C, H, W)
                      for i in range(N_CORES)])
    ab = np.stack([res.results[i]["out_ab"].reshape(2)
                   for i in range(N_CORES)])
    alpha = np.ascontiguousarray(ab[:, 0]).reshape(B, 1, 1, 1)
    beta = np.ascontiguousarray(ab[:, 1]).reshape(B, 1, 1, 1)
    return noisy, alpha, beta


# revision 14
# speedup vs baseline: 2.0069x; 1.4261x over previous
"""Trainium2 Bass kernel for nn_AdaptiveNoiseModule0 (8-core data parallel).

Math (per sample b):
  feat   = mean_L(bottleneck_feat[b])                  # [512]
  h      = relu(LN(feat @ W1 + b1) * ln_g + ln_b)
  a, be  = sigmoid(h @ W2 + b2) * 1.5e-4               # per-sample scalars
  sigma  = sqrt(a * x_bg + be)       (variance >= be ~ 7e-5 >> 1e-7 floor)
  n      = gaussian_blur(noise, 7, 0.05) * 1.5 == noise * 1.5
           (the 7-tap sigma=0.05 kernel underflows to the delta in f32)
  msoft  = gaussian_blur(mask, 21, 5.0)                # separable 21-tap
  out    = x_bg + n * sigma * msoft

Device mapping: one sample per NeuronCore. The 21-tap separable blur runs on
the TensorEngine as banded matmuls plus PE transposes
(T1 -> W-blur -> T2 -> H-blur) in bf16. The streaming phase computes
sigma' = sqrt(2.25*a*x + 2.25*be) on ScalarE (the 1.5 noise gain folded into
the radicand), noise*sigma' on GpSimd, and the rest on VectorE, with the
noise term in bf16 (it contributes ~1% of the output) and the dominant x_bg
term kept in f32. noise and mask are pre-cast to bf16 on the host (identical
rounding to the on-device cast they'd otherwise get) to cut HBM traffic.
SBUF slots of early-phase tiles are tag-shared with late-phase tiles so the
x/noise prefetch can run deep while the blur chain completes.
"""

import numpy as np
import ml_dtypes

B, C, H, W = 8, 3, 1024, 1024
L, D = 1024, 512
DH = 128
MAX_ALPHA = 0.00015
MAX_BETA = 0.00015
LN_EPS = 1e-5
KSIZE, SIGMA = 21, 5.0
R = KSIZE // 2
N_CORES = 8

_CACHE = {}


def _blur_weights():
    """Banded blur matrices for the 128-row-tile matmul formulation.

    out_tile[t] = Wprev.T @ tile[t-1] + Wmid.T @ tile[t] + Wnext.T @ tile[t+1]
    (edge tiles skip the missing neighbor == zero padding).
    """
    g = np.exp(-(np.linspace(-R, R, KSIZE, dtype=np.float32) ** 2)
               / (2.0 * np.float32(SIGMA) ** 2))
    g = (g / g.sum()).astype(np.float32)

    wmid = np.zeros((128, 128), np.float32)
    for kk in range(128):
        for m in range(max(0, kk - R), min(128, kk + R + 1)):
            wmid[kk, m] = g[kk - m + R]
    wprev = np.zeros((128, 128), np.float32)
    for kk in range(118, 128):
        for m in range(0, kk - 118 + 1):
            wprev[kk, m] = g[kk - 118 - m]
    wnext = np.zeros((128, 128), np.float32)
    for kk in range(10):
        for m in range(118 + kk, 128):
            wnext[kk, m] = g[138 + kk - m]
    bf = ml_dtypes.bfloat16
    return wmid.astype(bf), wprev.astype(bf), wnext.astype(bf)


def _build():
    import concourse.bass as bass
    import concourse.tile as tile
    from concourse import bacc, mybir
    from contextlib import ExitStack

    f32 = mybir.dt.float32
    bf16 = mybir.dt.bfloat16
    AF = mybir.ActivationFunctionType
    AX = mybir.AxisListType

    nc = bacc.Bacc("TRN2", target_bir_lowering=False, debug=False)

    # ---- per-core parameters (shard = one sample) ----
    x_d = nc.declare_dram_parameter("x", [C * H, W], f32, isOutput=False)
    nz_d = nc.declare_dram_parameter("nz", [C * H, W], bf16, isOutput=False)
    mk_d = nc.declare_dram_parameter("mk", [H, W], bf16, isOutput=False)
    bft_d = nc.declare_dram_parameter("bft", [D, L], f32, isOutput=False)
    w1_d = nc.declare_dram_parameter("w1s", [D, DH], f32, isOutput=False)
    b1_d = nc.declare_dram_parameter("b1c", [DH, 1], f32, isOutput=False)
    lng_d = nc.declare_dram_parameter("lngr", [1, DH], f32, isOutput=False)
    lnb_d = nc.declare_dram_parameter("lnbr", [1, DH], f32, isOutput=False)
    w2_d = nc.declare_dram_parameter("w2", [DH, 2], f32, isOutput=False)
    b2_d = nc.declare_dram_parameter("b2r", [1, 2], f32, isOutput=False)
    wmid_d = nc.declare_dram_parameter("wmid", [128, 128], bf16, isOutput=False)
    wprev_d = nc.declare_dram_parameter("wprev", [128, 128], bf16, isOutput=False)
    wnext_d = nc.declare_dram_parameter("wnext", [128, 128], bf16, isOutput=False)
    idf_d = nc.declare_dram_parameter("idf", [128, 128], f32, isOutput=False)
    idb_d = nc.declare_dram_parameter("idb", [128, 128], bf16, isOutput=False)
    ones_d = nc.declare_dram_parameter("onesr", [1, 128], f32, isOutput=False)

    out_d = nc.declare_dram_parameter("out", [C * H, W], f32, isOutput=True)
    ab_d = nc.declare_dram_parameter("out_ab", [1, 2], f32, isOutput=True)

    with tile.TileContext(nc) as tc, ExitStack() as es:
        def pool(name, bufs, space=None):
            kw = {"space": space} if space else {}
            return es.enter_context(tc.tile_pool(name=name, bufs=bufs, **kw))

        PSUM = bass.MemorySpace.PSUM
        consts = pool("consts", 1)
        headp = pool("headp", 2)
        featp = pool("featp", 4)
        w1p = pool("w1p", 4)
        # bft tiles (early) share slots with phase-B out tiles (late):
        # both [128,1024] f32.
        bfo = pool("bfo", 8)
        maskp = pool("maskp", 8)
        sgp = pool("sgp", 4)
        # xt tiles (die ~mid-blur) share slots with t2 tiles (born late)
        xt2 = pool("xt2", 8)
        vtp = pool("vtp", 8)
        vp = pool("vp", 8)
        msp = pool("msp", 8)
        xp = pool("xp", 8)
        nbp = pool("nbp", 8)
        t1p = pool("t1p", 4)
        ps_mm = pool("ps_mm", 3, PSUM)
        ps_tr = pool("ps_tr", 3, PSUM)
        ps_hd = pool("ps_hd", 2, PSUM)
        if True:
            # ---------- constant loads (gpsimd SWDGE ring) ----------
            idf = consts.tile([128, 128], f32)
            nc.gpsimd.dma_start(idf[:], idf_d[:])
            idb = consts.tile([128, 128], bf16)
            nc.gpsimd.dma_start(idb[:], idb_d[:])
            wmid = consts.tile([128, 128], bf16)
            nc.gpsimd.dma_start(wmid[:], wmid_d[:])
            wprev = consts.tile([128, 128], bf16)
            nc.gpsimd.dma_start(wprev[:], wprev_d[:])
            wnext = consts.tile([128, 128], bf16)
            nc.gpsimd.dma_start(wnext[:], wnext_d[:])
            ones_r = consts.tile([1, 128], f32)
            nc.gpsimd.dma_start(ones_r[:], ones_d[:])

            # ---------- mask DMA (blur critical path, sync ring) ----------
            mk_t = []
            for m in range(8):
                t = maskp.tile([128, W], bf16, tag="mk")
                nc.sync.dma_start(t[:], mk_d[128 * m:128 * (m + 1), :])
                mk_t.append(t)

            # ---------- param head ----------
            feat_c = []
            for dc in range(4):
                bt = bfo.tile([128, L], f32, tag="bfo")
                nc.gpsimd.dma_start(bt[:], bft_d[128 * dc:128 * (dc + 1), :])
                fc = featp.tile([128, 1], f32, tag="feat")
                nc.vector.reduce_sum(fc[:], bt[:], axis=AX.X)
                feat_c.append(fc)
            w1_t = []
            for dc in range(4):
                wt = w1p.tile([128, DH], f32, tag="w1")
                nc.gpsimd.dma_start(wt[:], w1_d[128 * dc:128 * (dc + 1), :])
                w1_t.append(wt)
            b1_t = headp.tile([DH, 1], f32)
            nc.gpsimd.dma_start(b1_t[:], b1_d[:])
            lng_t = headp.tile([1, DH], f32)
            nc.gpsimd.dma_start(lng_t[:], lng_d[:])
            lnb_t = headp.tile([1, DH], f32)
            nc.gpsimd.dma_start(lnb_t[:], lnb_d[:])
            w2_t = headp.tile([DH, 2], f32)
            nc.gpsimd.dma_start(w2_t[:], w2_d[:])
            b2_t = headp.tile([1, 2], f32)
            nc.gpsimd.dma_start(b2_t[:], b2_d[:])

            h_ps = ps_hd.tile([DH, 1], f32, tag="head")
            for dc in range(4):
                nc.tensor.matmul(h_ps[:], w1_t[dc][:], feat_c[dc][:],
                                 start=(dc == 0), stop=(dc == 3))
            h_sb = headp.tile([DH, 1], f32)
            nc.vector.tensor_add(h_sb[:], h_ps[:], b1_t[:])

            ht_ps = ps_hd.tile([1, DH], f32, tag="head")
            nc.tensor.transpose(ht_ps[:], h_sb[:], idf[:])
            ht = headp.tile([1, DH], f32)
            nc.scalar.copy(ht[:], ht_ps[:])

            musum = headp.tile([1, 1], f32)
            nc.vector.reduce_sum(musum[:], ht[:], axis=AX.X)
            mu = headp.tile([1, 1], f32)
            nc.scalar.mul(mu[:], musum[:], 1.0 / DH)
            ctr = headp.tile([1, DH], f32)
            nc.vector.tensor_scalar_sub(ctr[:], ht[:], mu[:])
            sq = headp.tile([1, DH], f32)
            nc.vector.tensor_mul(sq[:], ctr[:], ctr[:])
            vsum = headp.tile([1, 1], f32)
            nc.vector.reduce_sum(vsum[:], sq[:], axis=AX.X)
            eps_t = headp.tile([1, 1], f32)
            nc.vector.memset(eps_t[:], LN_EPS)
            sstd = headp.tile([1, 1], f32)
            nc.scalar.activation(sstd[:], vsum[:], AF.Sqrt,
                                 bias=eps_t[:], scale=1.0 / DH)
            rstd = headp.tile([1, 1], f32)
            nc.vector.reciprocal(rstd[:], sstd[:])
            hn = headp.tile([1, DH], f32)
            nc.vector.tensor_scalar_mul(hn[:], ctr[:], rstd[:])
            hn2 = headp.tile([1, DH], f32)
            nc.vector.tensor_mul(hn2[:], hn[:], lng_t[:])
            hn3 = headp.tile([1, DH], f32)
            nc.vector.tensor_add(hn3[:], hn2[:], lnb_t[:])
            hr = headp.tile([1, DH], f32)
            nc.vector.tensor_scalar_max(hr[:], hn3[:], 0.0)

            hcol_ps = ps_hd.tile([DH, 1], f32, tag="head")
            nc.tensor.transpose(hcol_ps[:], hr[:], idf[0:1, 0:1])
            hcol = headp.tile([DH, 1], f32)
            nc.scalar.copy(hcol[:], hcol_ps[:])

            p_ps = ps_hd.tile([1, 2], f32, tag="head")
            nc.tensor.matmul(p_ps[:], hcol[:], w2_t[:], start=True, stop=True)
            pb = headp.tile([1, 2], f32)
            nc.vector.tensor_add(pb[:], p_ps[:], b2_t[:])
            sg = headp.tile([1, 2], f32)
            nc.scalar.activation(sg[:], pb[:], AF.Sigmoid)
            ab_out = headp.tile([1, 2], f32)
            nc.scalar.mul(ab_out[:], sg[:], MAX_ALPHA)
            nc.scalar.dma_start(ab_d[:], ab_out[:])
            # 2.25 folds the n = 1.5*noise gain into the radicand
            sg2 = headp.tile([1, 2], f32)
            nc.scalar.mul(sg2[:], sg[:], 2.25 * MAX_ALPHA)
            ab_ps = ps_hd.tile([128, 2], f32, tag="head")
            nc.tensor.matmul(ab_ps[:], ones_r[:], sg2[:], start=True, stop=True)
            alpha_b = headp.tile([128, 1], f32)
            nc.scalar.copy(alpha_b[:], ab_ps[:, 0:1])
            beta_b = headp.tile([128, 1], f32)
            nc.scalar.copy(beta_b[:], ab_ps[:, 1:2])

            # ---------- mask blur: T1 -> W-blur -> T2 -> H-blur ----------
            # T1: XT = mask^T (PE transposes, bf16)
            xt_t = []
            for j in range(8):
                xt = xt2.tile([128, W], bf16, tag="xt2")
                for half in range(2):
                    tp = ps_tr.tile([128, 512], bf16, tag="trb")
                    for q in range(4):
                        m = half * 4 + q
                        nc.tensor.transpose(
                            tp[:, 128 * q:128 * (q + 1)],
                            mk_t[m][:, 128 * j:128 * (j + 1)], idb[:])
                    nc.scalar.copy(xt[:, 512 * half:512 * (half + 1)], tp[:])
                xt_t.append(xt)

            def banded(dst_tiles, src_tiles, dst_pool, tag):
                for j in range(8):
                    dst = dst_pool.tile([128, W], bf16, tag=tag)
                    for half in range(2):
                        bp = ps_mm.tile([128, 512], f32, tag="mm")
                        sl = slice(512 * half, 512 * (half + 1))
                        mms = []
                        if j > 0:
                            mms.append((wprev, src_tiles[j - 1][:, sl]))
                        mms.append((wmid, src_tiles[j][:, sl]))
                        if j < 7:
                            mms.append((wnext, src_tiles[j + 1][:, sl]))
                        for i, (wt, rhs) in enumerate(mms):
                            nc.tensor.matmul(bp[:], wt[:], rhs,
                                             start=(i == 0), stop=(i == len(mms) - 1))
                        nc.vector.tensor_copy(dst[:, sl], bp[:])
                    dst_tiles.append(dst)

            # W-blur on XT: VT = A @ XT  (= (X A)^T)
            vt_t = []
            banded(vt_t, xt_t, vtp, "vt")

            # T2: V = VT^T (PE transposes, bf16)
            v_t = []
            for r in range(8):
                v = vp.tile([128, W], bf16, tag="v")
                for half in range(2):
                    tp = ps_tr.tile([128, 512], bf16, tag="trb")
                    for q in range(4):
                        m = half * 4 + q
                        nc.tensor.transpose(
                            tp[:, 128 * q:128 * (q + 1)],
                            vt_t[m][:, 128 * r:128 * (r + 1)], idb[:])
                    nc.scalar.copy(v[:, 512 * half:512 * (half + 1)], tp[:])
                v_t.append(v)

            # H-blur: msoft = A @ V
            ms_t = []
            banded(ms_t, v_t, msp, "ms")

            # ---------- streaming phase ----------
            for r in range(8):
                for c in range(C):
                    off = c * H + 128 * r
                    xt_ = xp.tile([128, W], f32, tag="x")
                    nc.sync.dma_start(xt_[:], x_d[off:off + 128, :])
                    nb_ = nbp.tile([128, W], bf16, tag="nb")
                    nc.sync.dma_start(nb_[:], nz_d[off:off + 128, :])
                    sg_ = sgp.tile([128, W], bf16, tag="sg")
                    nc.scalar.activation(sg_[:], xt_[:], AF.Sqrt,
                                         bias=beta_b[:], scale=alpha_b[:])
                    t1_ = t1p.tile([128, W], bf16, tag="t1")
                    nc.vector.tensor_mul(t1_[:], nb_[:], sg_[:])
                    t2_ = xt2.tile([128, W], bf16, tag="xt2")
                    nc.vector.tensor_mul(t2_[:], t1_[:], ms_t[r][:])
                    o_ = bfo.tile([128, W], f32, tag="bfo")
                    nc.vector.tensor_add(o_[:], xt_[:], t2_[:])
                    nc.scalar.dma_start(out_d[off:off + 128, :], o_[:])

    nc.compile()
    return nc


def _get_nc():
    if "nc" not in _CACHE:
        _CACHE["nc"] = _build()
    return _CACHE["nc"]


def kernel(x_bg, bottleneck_feat, mask, noise, W1, b1, ln_g, ln_b, W2, b2):
    from concourse.bass_utils import run_bass_kernel_spmd

    nc = _get_nc()
    bf = ml_dtypes.bfloat16

    wmid, wprev, wnext = _blur_weights()
    idf = np.eye(128, dtype=np.float32)
    shared = {
        "w1s": np.ascontiguousarray(W1.astype(np.float32) / np.float32(L)),
        "b1c": np.ascontiguousarray(b1.astype(np.float32).reshape(DH, 1)),
        "lngr": np.ascontiguousarray(ln_g.astype(np.float32).reshape(1, DH)),
        "lnbr": np.ascontiguousarray(ln_b.astype(np.float32).reshape(1, DH)),
        "w2": np.ascontiguousarray(W2.astype(np.float32)),
        "b2r": np.ascontiguousarray(b2.astype(np.float32).reshape(1, 2)),
        "wmid": wmid, "wprev": wprev, "wnext": wnext,
        "idf": idf, "idb": idf.astype(bf), "onesr": np.ones((1, 128), np.float32),
    }
    in_maps = []
    for i in range(N_CORES):
        in_maps.append({
            "x": np.ascontiguousarray(x_bg[i].reshape(C * H, W), dtype=np.float32),
            "nz": np.asarray(noise[i].reshape(C * H, W), dtype=bf),
            "mk": np.asarray(mask[i, 0], dtype=bf),
            "bft": np.ascontiguousarray(bottleneck_feat[i].T, dtype=np.float32),
            **shared,
        })

    res = run_bass_kernel_spmd(nc, in_maps, core_ids=list(range(N_CORES)))
    noisy = np.stack([res.results[i]["out"].reshape(C, H, W)
                      for i in range(N_CORES)])
    ab = np.stack([res.results[i]["out_ab"].reshape(2)
                   for i in range(N_CORES)])
    alpha = np.ascontiguousarray(ab[:, 0]).reshape(B, 1, 1, 1)
    beta = np.ascontiguousarray(ab[:, 1]).reshape(B, 1, 1, 1)
    return noisy, alpha, beta
